# revision 4
# baseline (speedup 1.0000x reference)
"""Trainium2 Bass kernel: BinarizeLinear inference.

Computes out = sign01(x) @ weight + bias where sign01(t) = +1 if t > 0 else -1,
for x [8192, 4096] f32, weight [4096, 4096] f32, bias [4096] f32.

Strategy: 2D shard across 8 NeuronCores as a 4 (token) x 2 (out-feature)
grid — per-core m=2048, n=2048, k=4096 — minimizing per-core HBM traffic.
No collectives; outputs are assembled on the host.

Per-core kernel:
  - stage x (host-pretransposed to [k, m]) and weight into SBUF as bf16 via
    SWDGE cast-DMA (halves DMA bus time vs f32; sign(x) is unaffected by
    bf16 rounding, and the weight's bf16 rounding is ~2^-9 relative),
  - binarize x to fp8e4 {+1,-1} on the Scalar engine (Sign activation with
    a tiny negative bias so exact zeros map to -1 like the reference);
    kept resident in SBUF (4 MB),
  - split each 512-column weight chunk on-chip into hi = fp8e4(w) (Scalar
    cast) and lo = fp8e4(w - hi) (Vector subtract with fp8 output cast):
    hi + lo ~= w to ~2^-8 relative,
  - matmul with MatmulPerfMode.DoubleRow (both operands fp8e4, 2 k-tiles of
    128 per instruction, 0.5 cycles/output-row on TRN2): 2x bf16 matmul
    throughput even with the hi+lo double pass, at bf16-level accuracy,
  - accumulate out tiles [128m, 512n] in PSUM over 32 DoubleRow matmuls
    (hi pass + lo pass), evict with a fused bias add on Vector, DMA to DRAM.
"""

import contextlib
import os
import sys

import numpy as np

os.environ.setdefault("JAX_PLATFORMS", "axon")

for _p in ("/opt/trn_rl_repo", "/root/.axon_site/_ro/trn_rl_repo"):
    if os.path.isdir(_p) and _p not in sys.path:
        sys.path.insert(0, _p)
        break

import concourse.bass as bass  # noqa: E402
import concourse.mybir as mybir  # noqa: E402
import concourse.tile as tile  # noqa: E402
from concourse import bacc  # noqa: E402
from concourse.bass_utils import run_bass_kernel_spmd  # noqa: E402

P = 128
N_CORES = 8
TOKENS, IN_F, OUT_F = 8192, 4096, 4096
R, C = 4, 2  # token-way x out-feature-way core grid
M_SHARD = TOKENS // R  # 2048
N_SHARD = OUT_F // C  # 2048
F32 = mybir.dt.float32
BF16 = mybir.dt.bfloat16
FP8 = mybir.dt.float8e4
DR = mybir.MatmulPerfMode.DoubleRow


def build_nc(
    m_shard=M_SHARD,
    k=IN_F,
    n=N_SHARD,
    n_chunk=512,
    mb=256,
    ktg=4,
    loop_k=1,
):
    """loop_k > 1 wraps the whole body in a hardware For loop that repeats
    the identical computation; used only for wall-clock slope timing.
    ktg = k-tiles per staged DMA/convert instruction (batching)."""
    mt_n = m_shard // P
    kt_n = k // P
    kt2_n = kt_n // 2
    nt_n = n // n_chunk
    mb_n = m_shard // mb
    mt_per_mb = mb // P
    assert m_shard % mb == 0 and mb % P == 0 and k % (2 * P) == 0
    assert n % n_chunk == 0 and kt_n % ktg == 0

    nc = bacc.Bacc(
        "TRN2", target_bir_lowering=False, debug=False, num_devices=N_CORES
    )
    # x arrives host-pretransposed as [k, m_shard]
    x_ap = nc.declare_dram_parameter("x", [k, m_shard], F32, isOutput=False).ap()
    w_ap = nc.declare_dram_parameter("weight", [k, n], F32, isOutput=False).ap()
    b_ap = nc.declare_dram_parameter("bias", [P, n], F32, isOutput=False).ap()
    out_ap = nc.declare_dram_parameter("out", [m_shard, n], F32, isOutput=True).ap()
    # weight rows k = kt*P + p -> [p, kt, n]; same for x
    w_t = w_ap.rearrange("(kt p) n -> p kt n", p=P)
    xt_t = x_ap.rearrange("(kt p) m -> p kt m", p=P)

    with tile.TileContext(nc) as tc:
        with (
            tc.tile_pool(name="const", bufs=1) as const_pool,
            tc.tile_pool(name="xbt", bufs=1) as xbt_pool,
            tc.tile_pool(name="xstage", bufs=4) as xstage_pool,
            tc.tile_pool(name="wstage", bufs=4) as wstage_pool,
            tc.tile_pool(name="wchunk", bufs=2) as w_pool,
            tc.tile_pool(name="osb", bufs=4) as o_pool,
            tc.tile_pool(name="mm_psum", bufs=6, space="PSUM") as mm_psum,
        ):
            bias_sb = const_pool.tile([P, n], F32)
            nc.sync.dma_start(bias_sb[:], b_ap[:, :])
            # per-partition tiny negative bias for the sign-binarize
            sgn_bias = const_pool.tile([P, 1], F32)
            nc.gpsimd.memset(sgn_bias[:], -1e-30)

            loop_cm = (
                tc.For_i(0, loop_k, 1) if loop_k > 1 else contextlib.nullcontext()
            )
            with loop_cm:
                # Binarized-transposed activations, resident: [P(k), kt, m]
                xbt = xbt_pool.tile([P, kt_n, m_shard], FP8)

                def binarize_mb(mbi):
                    m_bl = slice(mbi * mb, (mbi + 1) * mb)
                    for kt in range(0, kt_n, ktg):
                        kg = slice(kt, kt + ktg)
                        xr = xstage_pool.tile([P, ktg, mb], BF16)
                        nc.gpsimd.dma_start(xr[:], xt_t[:, kg, m_bl])
                        # sign(x - tiny): zeros -> -1, matching where(x>0,1,-1)
                        nc.scalar.sign(xbt[:, kg, m_bl], xr[:], bias=sgn_bias[:])

                whi = [None] * nt_n
                wlo = [None] * nt_n

                def load_wchunk(nt):
                    n_sl = slice(nt * n_chunk, (nt + 1) * n_chunk)
                    whi[nt] = w_pool.tile([P, kt_n, n_chunk], FP8, tag="whi", name="whi")
                    wlo[nt] = w_pool.tile([P, kt_n, n_chunk], FP8, tag="wlo", name="wlo")
                    for kt in range(0, kt_n, ktg):
                        kg = slice(kt, kt + ktg)
                        wst = wstage_pool.tile([P, ktg, n_chunk], BF16)
                        nc.gpsimd.dma_start(wst[:], w_t[:, kg, n_sl])
                        nc.scalar.activation(
                            whi[nt][:, kg, :],
                            wst[:],
                            mybir.ActivationFunctionType.Copy,
                        )
                        nc.vector.tensor_tensor(
                            wlo[nt][:, kg, :],
                            wst[:],
                            whi[nt][:, kg, :],
                            mybir.AluOpType.subtract,
                        )

                def do_cell(mt, nt):
                    # one out tile [P, n_chunk]: hi pass + lo pass, DoubleRow
                    m_sl = slice(mt * P, (mt + 1) * P)
                    n_sl = slice(nt * n_chunk, (nt + 1) * n_chunk)
                    ps = mm_psum.tile([P, n_chunk], F32)
                    for wc, first, last in (
                        (whi[nt], True, False),
                        (wlo[nt], False, True),
                    ):
                        for k2 in range(kt2_n):
                            ksl = slice(2 * k2, 2 * k2 + 2)
                            nc.tensor.matmul(
                                ps[:],
                                xbt[:, ksl, m_sl],
                                wc[:, ksl, :],
                                start=first and k2 == 0,
                                stop=last and k2 == kt2_n - 1,
                                perf_mode=DR,
                            )
                    osb = o_pool.tile([P, n_chunk], F32)
                    nc.vector.tensor_add(osb[:], ps[:], bias_sb[:, n_sl])
                    nc.sync.dma_start(out_ap[m_sl, n_sl], osb[:])

                # Emission order = per-engine issue order. Load w chunk 0,
                # stream all of x (bf16: ~46us), then go column-major; chunk
                # nt+1 loads during column nt's matmuls (w_pool bufs=2).
                load_wchunk(0)
                for mbi in range(mb_n):
                    binarize_mb(mbi)
                for nt in range(nt_n):
                    if nt + 1 < nt_n:
                        load_wchunk(nt + 1)
                    for mt in range(mt_n):
                        do_cell(mt, nt)

    nc.compile()
    return nc


def shard_inputs(x, weight, bias):
    """Host-side sharding for the 4x2 grid; core = ti*C + ni."""
    xt = np.ascontiguousarray(x.T)  # [k, tokens]
    x_shards = [
        np.ascontiguousarray(xt[:, ti * M_SHARD : (ti + 1) * M_SHARD])
        for ti in range(R)
    ]
    w_shards = [
        np.ascontiguousarray(weight[:, ni * N_SHARD : (ni + 1) * N_SHARD])
        for ni in range(C)
    ]
    b_shards = [
        np.ascontiguousarray(
            np.broadcast_to(
                bias[None, ni * N_SHARD : (ni + 1) * N_SHARD], (P, N_SHARD)
            )
        )
        for ni in range(C)
    ]
    return [
        {"x": x_shards[c // C], "weight": w_shards[c % C], "bias": b_shards[c % C]}
        for c in range(N_CORES)
    ]


def unshard_output(outs):
    return np.concatenate(
        [
            np.concatenate([outs[ti * C + ni] for ni in range(C)], axis=1)
            for ti in range(R)
        ],
        axis=0,
    )


_NC_CACHE = {}


def _get_nc(cfg):
    nc = _NC_CACHE.get(cfg)
    if nc is None:
        nc = _NC_CACHE[cfg] = build_nc(*cfg)
    return nc


def kernel(x, weight, bias, _trace=False):
    x = np.ascontiguousarray(np.asarray(x, dtype=np.float32))
    weight = np.ascontiguousarray(np.asarray(weight, dtype=np.float32))
    bias = np.ascontiguousarray(np.asarray(bias, dtype=np.float32))
    assert x.shape == (TOKENS, IN_F) and weight.shape == (IN_F, OUT_F)

    in_maps = shard_inputs(x, weight, bias)
    nc = _get_nc((M_SHARD, IN_F, N_SHARD, 512, 256, 4, 1))
    res = run_bass_kernel_spmd(nc, in_maps, list(range(N_CORES)), trace=_trace)
    out = unshard_output([res.results[c]["out"] for c in range(N_CORES)])
    if _trace:
        return out, res
    return out
